# revision 1
# baseline (speedup 1.0000x reference)
"""nn_DBS_lstm on 8 trn2 NeuronCores.

3-layer LSTM (T=512, H=1024, batch=1) + 20670-dim output projection.

Strategy: the recurrent scan is latency-bound and cannot tolerate per-step
cross-core communication (collective latency ~10us >> step time), so each
LSTM layer's scan runs whole on one core (cores 0/1/2), software-pipelined
across cores with a one-block skew. Cross-core traffic is one AllGather
("mailbox") per block-slot carrying the h-blocks produced in the previous
slot; each mailbox is consumed only at the END of the next slot (after that
slot's scan), so every collective completes under ~250us of scan and no core
ever waits on one. Lags compound per hop: core p scans block s-(2p+1). The
input projections (x @ W_ih^T) for layers 1/2 are computed per-block as GEMMs
from mailbox data; layer 0's comes from the (static) pose input via per-core
data masks. The final hidden2dbs GEMM is sharded across all 8 cores.

Numerics: weights/h in bf16 (PE fast-weight-load), PSUM/elementwise in fp32.
Sigmoid is computed via tanh with 0.5-prescaled rows (ACT does one tanh over
all 4 gate blocks); h is stored doubled (h' = 2h) with 0.5 folded into every
weight column that consumes h.
"""

import time

import numpy as np

import concourse.bass as bass
import concourse.mybir as mybir
import concourse.tile as tile
from concourse import bacc
from concourse.bass import ds

try:
    import ml_dtypes

    _BF16 = ml_dtypes.bfloat16
except Exception:  # pragma: no cover
    _BF16 = np.float32

T = 512
IN_DIM = 85
H = 1024
KC = H // 128            # 8 k-chunks
M = 4 * H                # 4096 gates
MT = M // 128            # 32 m-tiles
LAYERS = 3
VERTS = 6890
NOUT = VERTS * 3         # 20670
NCORES = 8
NSLICE = 2584            # padded per-core output slice (8*2584 = 20672)
B = 16                   # steps per block-slot
NB = T // B              # 32 blocks
NS = NB + 6              # slots: core p scans block s-(2p+1); mailbox read at slot end

f32 = mybir.dt.float32
bf16 = mybir.dt.bfloat16
i32 = mybir.dt.int32
AF = mybir.ActivationFunctionType
ALU = mybir.AluOpType


def build():
    nc = bacc.Bacc("TRN2", target_bir_lowering=False, debug=False)

    whh_d = nc.dram_tensor("whh", [128, KC * M], bf16, kind="ExternalInput")
    wih_d = nc.dram_tensor("wih", [128, KC * M], bf16, kind="ExternalInput")
    bias_d = nc.dram_tensor("bias", [128, MT], f32, kind="ExternalInput")
    pose_d = nc.dram_tensor("pose", [128, 8 * T], bf16, kind="ExternalInput")
    wout_d = nc.dram_tensor("wout", [128, KC * NSLICE], bf16, kind="ExternalInput")
    masks_d = nc.dram_tensor("masks", [128, 8], f32, kind="ExternalInput")
    out_d = nc.dram_tensor("out", [T, NSLICE], f32, kind="ExternalOutput")

    SCR = 8 * (NS * B + 1)   # h-history cols (slot-indexed, +1 for final h)

    with tile.TileContext(nc) as tc:
        with (
            tc.tile_pool(name="persist", bufs=1) as pers,
            tc.tile_pool(name="dram", bufs=1, space="DRAM") as dp,
        ):
            whh = pers.tile([128, KC * M], bf16)
            wih = pers.tile([128, KC * M], bf16)
            bia = pers.tile([128, MT], f32)
            msk = pers.tile([128, 8], f32)
            pose = pers.tile([128, 8 * T], bf16)
            nc.gpsimd.dma_start(whh[:], whh_d[:])
            nc.gpsimd.dma_start(wih[:], wih_d[:])
            nc.gpsimd.dma_start(bia[:], bias_d[:])
            nc.gpsimd.dma_start(msk[:], masks_d[:])
            nc.gpsimd.dma_start(pose[:], pose_d[:])

            hh = pers.tile([128, SCR], bf16)       # h' history, slot-indexed
            h2 = pers.tile([128, 8 * T], bf16)     # layer-2 h' history (all cores)
            cs = pers.tile([128, 8], f32)          # c' state
            zb = pers.tile([128, 8], bf16)
            zf = pers.tile([128, 8], f32)
            xgb = pers.tile([128, 32 * B], f32)    # current block's gate inputs
            nc.vector.memset(hh[:, 0:8], 0.0)
            nc.vector.memset(xgb[:], 0.0)
            nc.vector.memset(cs[:], 0.0)
            nc.vector.memset(zb[:], 0.0)
            nc.vector.memset(zf[:], 0.0)

            # mailbox bounce buffers + DRAM zero sources for conditional resets
            obox = dp.tile([128, 8 * B], bf16)
            iboxes = {
                s: dp.tile(
                    [8 * 128, 8 * B], bf16, addr_space="Shared", name=f"ibox{s}"
                )
                for s in range(1, NS)
            }

            with (
                tc.tile_pool(name="stage", bufs=2) as stg,
                tc.tile_pool(name="ew", bufs=2) as ep,
                tc.tile_pool(name="psA", bufs=2, space="PSUM") as ppg,
                tc.tile_pool(name="psB", bufs=2, space="PSUM") as ppx,
            ):

                def xg_gemm(st_ap):
                    """gate inputs for one block: xgb[:, t*32+m] from staging."""
                    for m in range(MT):
                        px = ppx.tile([128, B], f32, tag="xg")
                        for k in range(KC):
                            nc.tensor.matmul(
                                px[:],
                                wih[:, k * M + m * 128 : k * M + (m + 1) * 128],
                                st_ap[:, ds(k, B, 8)],
                                start=(k == 0),
                                stop=(k == KC - 1),
                            )
                        nc.vector.tensor_scalar_add(
                            xgb[:, ds(m, B, 32)], px[:], bia[:, m : m + 1]
                        )

                def scan_step(iv, base):
                    """one LSTM step; reads h at base+iv*8, writes +8."""
                    ps = ppg.tile([128, MT], f32, tag="gates")
                    for m in range(MT):
                        for k in range(KC):
                            nc.tensor.matmul(
                                ps[:, m : m + 1],
                                whh[:, k * M + m * 128 : k * M + (m + 1) * 128],
                                hh[:, ds(iv * 8 + (base + k), 1)],
                                start=(k == 0),
                                stop=(k == KC - 1),
                            )
                    pre = ep.tile([128, MT], f32, tag="pre")
                    nc.vector.tensor_add(pre[:], ps[:], xgb[:, ds(iv * 32, 32)])
                    th = ep.tile([128, MT], f32, tag="th")
                    nc.scalar.activation(th[:], pre[:], AF.Tanh)
                    # cols 0:8 i, 8:16 f, 16:24 o, 24:32 g
                    A = ep.tile([128, 8], f32, tag="A")
                    nc.vector.scalar_tensor_tensor(
                        A[:], th[:, 8:16], 1.0, cs[:], ALU.add, ALU.mult
                    )
                    Bt = ep.tile([128, 8], f32, tag="B")
                    nc.vector.scalar_tensor_tensor(
                        Bt[:], th[:, 0:8], 1.0, th[:, 24:32], ALU.add, ALU.mult
                    )
                    nc.vector.scalar_tensor_tensor(
                        cs[:], A[:], 0.5, Bt[:], ALU.mult, ALU.add
                    )
                    tch = ep.tile([128, 8], f32, tag="tc")
                    nc.scalar.activation(tch[:], cs[:], AF.Tanh, scale=0.5)
                    nc.vector.scalar_tensor_tensor(
                        hh[:, ds(iv * 8 + (base + 8), 8)],
                        th[:, 16:24], 1.0, tch[:], ALU.add, ALU.mult,
                    )

                for s in range(NS):
                    if s >= 1:
                        # ship previous slot's products; consumers read this
                        # mailbox only at the END of this slot, so the CC
                        # completes under the scan and nobody waits on it
                        nc.gpsimd.dma_start(
                            obox[:], hh[:, (s - 1) * B * 8 + 8 : s * B * 8 + 8]
                        )
                        nc.gpsimd.collective_compute(
                            "AllGather",
                            ALU.bypass,
                            replica_groups=[list(range(NCORES))],
                            ins=[obox[:].opt()],
                            outs=[iboxes[s][:].opt()],
                        )
                    # state resets just before each core's first valid block
                    # (core p scans block 0 at slot 2p+1)
                    if s in (1, 3, 5):
                        km = {1: msk[:, 4:5], 3: msk[:, 5:6], 5: msk[:, 6:7]}[s]
                        nc.vector.tensor_scalar_mul(cs[:], cs[:], km)
                        nc.vector.tensor_scalar_mul(
                            hh[:, s * B * 8 : s * B * 8 + 8],
                            hh[:, s * B * 8 : s * B * 8 + 8], km,
                        )
                    # scan this slot's block with xgb prepared last slot
                    if s <= NB + 4:
                        base = s * B * 8
                        with tc.For_i(0, B, 1, hint_engines=(mybir.EngineType.PE,), staggered_reset=True) as iv:
                            scan_step(iv, base)
                    # collect layer-2 h' blocks for the output GEMM
                    if 6 <= s <= NB + 5:
                        nc.gpsimd.dma_start(
                            h2[:, (s - 6) * B * 8 : (s - 5) * B * 8],
                            iboxes[s][2 * 128 : 3 * 128, :],
                        )
                    # prepare NEXT slot's gate inputs from this slot's mailbox
                    if s <= NB + 3:
                        st = stg.tile([128, 8 * B], bf16, tag="st")
                        if s >= 1:
                            r0 = stg.tile([128, 8 * B], bf16, tag="r0")
                            r1 = stg.tile([128, 8 * B], bf16, tag="r1")
                            nc.gpsimd.dma_start(r0[:], iboxes[s][0:128, :])
                            nc.gpsimd.dma_start(r1[:], iboxes[s][128:256, :])
                            nc.vector.tensor_scalar_mul(st[:], r0[:], msk[:, 0:1])
                            nc.vector.scalar_tensor_tensor(
                                st[:], r1[:], msk[:, 1:2], st[:], ALU.mult, ALU.add
                            )
                            if s < NB:
                                nc.vector.scalar_tensor_tensor(
                                    st[:], pose[:, s * 8 * B : (s + 1) * 8 * B],
                                    msk[:, 3:4], st[:], ALU.mult, ALU.add,
                                )
                        else:
                            nc.vector.tensor_scalar_mul(
                                st[:], pose[:, 0 : 8 * B], msk[:, 3:4]
                            )
                        xg_gemm(st)

            # ---- output GEMM: out[t, n] = h2'[1..T] @ wout_eff^T ----
            with (
                tc.tile_pool(name="og_sb", bufs=3) as osb,
                tc.tile_pool(name="og_ps", bufs=2, space="PSUM") as ops,
            ):
                nchunks = [
                    (i * 512, min(512, NSLICE - i * 512))
                    for i in range((NSLICE + 511) // 512)
                ]
                for n0, nsz in nchunks:
                    psl = [
                        ops.tile([128, 512], f32, tag=f"og{tt}", name=f"og{tt}")
                        for tt in range(T // 128)
                    ]
                    for k in range(KC):
                        w = osb.tile([128, 512], bf16, tag="wo")
                        nc.gpsimd.dma_start(
                            w[:, :nsz],
                            wout_d[:, k * NSLICE + n0 : k * NSLICE + n0 + nsz],
                        )
                        for tt in range(T // 128):
                            nc.tensor.matmul(
                                psl[tt][:, :nsz],
                                h2[:, ds(tt * 1024 + k, 128, 8)],
                                w[:, :nsz],
                                start=(k == 0),
                                stop=(k == KC - 1),
                            )
                    for tt in range(T // 128):
                        ot = osb.tile([128, 512], f32, tag="ot")
                        nc.scalar.copy(ot[:, :nsz], psl[tt][:, :nsz])
                        nc.gpsimd.dma_start(
                            out_d[tt * 128 : (tt + 1) * 128, n0 : n0 + nsz],
                            ot[:, :nsz],
                        )

    nc.compile()
    return nc


def build_baseline():
    """Same I/O signature, near-empty body — measures dispatch overhead."""
    nc = bacc.Bacc("TRN2", target_bir_lowering=False, debug=False)
    nc.dram_tensor("whh", [128, KC * M], bf16, kind="ExternalInput")
    nc.dram_tensor("wih", [128, KC * M], bf16, kind="ExternalInput")
    nc.dram_tensor("bias", [128, MT], f32, kind="ExternalInput")
    nc.dram_tensor("pose", [128, 8 * T], bf16, kind="ExternalInput")
    nc.dram_tensor("wout", [128, KC * NSLICE], bf16, kind="ExternalInput")
    masks_d = nc.dram_tensor("masks", [128, 8], f32, kind="ExternalInput")
    out_d = nc.dram_tensor("out", [T, NSLICE], f32, kind="ExternalOutput")
    with tile.TileContext(nc) as tc:
        with tc.tile_pool(name="sb", bufs=1) as sb:
            t = sb.tile([128, 8], f32)
            nc.gpsimd.dma_start(t[:], masks_d[:])
            nc.gpsimd.dma_start(out_d[0:128, 0:8], t[:])
    nc.compile()
    return nc


_CACHE = {}


def _prep_core_inputs(inputs):
    """Slice/scale/reorder reference weights into per-core device layouts."""
    pose_np = np.asarray(inputs["pose_beta_seq"], np.float32)      # [T, 85]
    w_out = np.asarray(inputs["W_out"], np.float32)                # [20670, H]

    def to_kc_m(WT, m_cols):
        # WT: [H, m_cols] -> [128, KC*m_cols]
        o = np.empty((128, KC * m_cols), np.float32)
        for k in range(KC):
            o[:, k * m_cols : (k + 1) * m_cols] = WT[k * 128 : (k + 1) * 128, :]
        return o

    def reorder_rows(Wr):
        i, f, g, o = np.split(Wr, 4, axis=0)
        return np.concatenate([i, f, o, g], axis=0)

    rowscale = np.ones((M, 1), np.float32)
    rowscale[: 3 * H] = 0.5      # i,f,o rows -> tanh-as-sigmoid

    per_core = []
    for p in range(NCORES):
        d = {}
        if p < LAYERS:
            Whh = reorder_rows(np.asarray(inputs[f"W_hh_{p}"], np.float32))
            Wih = reorder_rows(np.asarray(inputs[f"W_ih_{p}"], np.float32))
            bi = np.asarray(inputs[f"b_ih_{p}"], np.float32) + np.asarray(
                inputs[f"b_hh_{p}"], np.float32
            )
            bi = reorder_rows(bi.reshape(M, 1))
            in_dim = Wih.shape[1]
            Wih_p = np.zeros((M, H), np.float32)
            colscale = 0.5 if p > 0 else 1.0     # inputs are h' = 2h for l>0
            Wih_p[:, :in_dim] = Wih * colscale
            whh_eff = Whh * rowscale * 0.5       # own-state h' = 2h
            wih_eff = Wih_p * rowscale
            b_eff = bi * rowscale
            d["whh"] = to_kc_m(whh_eff.T, M).astype(_BF16)
            d["wih"] = to_kc_m(wih_eff.T, M).astype(_BF16)
            d["bias"] = b_eff.reshape(MT, 128).T.astype(np.float32)
        else:
            d["whh"] = np.zeros((128, KC * M), _BF16)
            d["wih"] = np.zeros((128, KC * M), _BF16)
            d["bias"] = np.zeros((128, MT), np.float32)
        if p == 0:
            pT = np.zeros((H, T), np.float32)
            pT[:IN_DIM, :] = pose_np.T
            po = pT.reshape(KC, 128, T).transpose(1, 2, 0).reshape(128, 8 * T)
            d["pose"] = po.astype(_BF16)
        else:
            d["pose"] = np.zeros((128, 8 * T), _BF16)
        wo = np.zeros((NSLICE, H), np.float32)
        lo = p * NSLICE
        hi = min(NOUT, lo + NSLICE)
        if hi > lo:
            wo[: hi - lo] = w_out[lo:hi]
        d["wout"] = to_kc_m((wo * 0.5).T, NSLICE).astype(_BF16)
        mk = np.zeros((8,), np.float32)
        mk[0] = 1.0 if p == 1 else 0.0   # use mailbox rank 0
        mk[1] = 1.0 if p == 2 else 0.0   # use mailbox rank 1
        mk[3] = 1.0 if p == 0 else 0.0   # use pose
        mk[4] = 0.0 if p == 0 else 1.0   # reset at slot 1 (core 0)
        mk[5] = 0.0 if p == 1 else 1.0   # reset at slot 3 (core 1)
        mk[6] = 0.0 if p == 2 else 1.0   # reset at slot 5 (core 2)
        d["masks"] = np.tile(mk[None, :], (128, 1))
        per_core.append(d)
    return per_core


def _get_runner(which="full"):
    key = f"runner_{which}"
    if key in _CACHE:
        return _CACHE[key]
    import jax
    from jax.sharding import Mesh, PartitionSpec
    from jax.experimental.shard_map import shard_map
    from concourse import bass2jax
    from concourse.bass2jax import _bass_exec_p, partition_id_tensor

    nc = build() if which == "full" else build_baseline()
    bass2jax.install_neuronx_cc_hook()

    in_names, out_names, out_avals, zero_outs = [], [], [], []
    partition_name = nc.partition_id_tensor.name if nc.partition_id_tensor else None
    for alloc in nc.m.functions[0].allocations:
        if not isinstance(alloc, mybir.MemoryLocationSet):
            continue
        name = alloc.memorylocations[0].name
        if alloc.kind == "ExternalInput":
            if name != partition_name:
                in_names.append(name)
        elif alloc.kind == "ExternalOutput":
            out_names.append(name)
            shape = tuple(alloc.tensor_shape)
            dtype = mybir.dt.np(alloc.dtype)
            out_avals.append(jax.core.ShapedArray(shape, dtype))
            zero_outs.append(np.zeros(shape, dtype))
    n_params = len(in_names)
    all_in = list(in_names) + list(out_names)
    if partition_name is not None:
        all_in.append(partition_name)

    def _body(*args):
        operands = list(args)
        if partition_name is not None:
            operands.append(partition_id_tensor())
        outs = _bass_exec_p.bind(
            *operands,
            out_avals=tuple(out_avals),
            in_names=tuple(all_in),
            out_names=tuple(out_names),
            lowering_input_output_aliases=(),
            sim_require_finite=True,
            sim_require_nnan=True,
            nc=nc,
        )
        return tuple(outs)

    devices = jax.devices()[:NCORES]
    mesh = Mesh(np.asarray(devices), ("core",))
    nouts = len(out_names)
    sharded = jax.jit(
        shard_map(
            _body,
            mesh=mesh,
            in_specs=(PartitionSpec("core"),) * (n_params + nouts),
            out_specs=(PartitionSpec("core"),) * nouts,
            check_rep=False,
        ),
        donate_argnums=tuple(range(n_params, n_params + nouts)),
    )

    def run(per_core):
        gin = [
            np.concatenate([np.asarray(pc[nm]) for pc in per_core], axis=0)
            for nm in in_names
        ]
        gzo = [np.concatenate([z] * NCORES, axis=0) for z in zero_outs]
        sh = jax.sharding.NamedSharding(mesh, PartitionSpec("core"))
        gin = [jax.device_put(a, sh) for a in gin]
        gzo = [jax.device_put(a, sh) for a in gzo]
        jax.block_until_ready(gin)
        t0 = time.time()
        outs = sharded(*gin, *gzo)
        jax.block_until_ready(outs)
        _CACHE[f"exec_wall_s_{which}"] = time.time() - t0
        res = [np.asarray(o) for o in outs]
        per_core_out = []
        for c in range(NCORES):
            per_core_out.append(
                {
                    nm: res[i][
                        c * zero_outs[i].shape[0] : (c + 1) * zero_outs[i].shape[0]
                    ]
                    for i, nm in enumerate(out_names)
                }
            )
        return per_core_out

    _CACHE[key] = run
    return run


def kernel(**inputs):
    run = _get_runner("full")
    per_core = _prep_core_inputs(inputs)
    t0 = time.time()
    outs = run(per_core)
    _CACHE["last_wall_s"] = time.time() - t0
    _CACHE["exec_wall_s"] = _CACHE.get("exec_wall_s_full")
    full = np.concatenate([outs[c]["out"] for c in range(NCORES)], axis=1)[:, :NOUT]
    return full.reshape(T, VERTS, 3).astype(np.float32)



# revision 6
# speedup vs baseline: 4.6218x; 4.6218x over previous
"""nn_DBS_lstm on 8 trn2 NeuronCores.

3-layer LSTM (T=512, H=1024, batch=1) + 20670-dim output projection.

Strategy: the recurrent scan is latency-bound and cannot tolerate per-step
cross-core communication (collective latency ~10us >> step time), so each
LSTM layer's scan runs whole on one core (cores 0/1/2), software-pipelined
across cores with a one-block skew. Cross-core traffic is one AllGather
("mailbox") per block-slot carrying the h-blocks produced in the previous
slot; each mailbox is consumed only at the END of the next slot (after that
slot's scan), so every collective completes under ~250us of scan and no core
ever waits on one. Lags compound per hop: core p scans block s-(2p+1). The
input projections (x @ W_ih^T) for layers 1/2 are computed per-block as GEMMs
from mailbox data; layer 0's comes from the (static) pose input via per-core
data masks. The final hidden2dbs GEMM is sharded across all 8 cores.

Numerics: weights/h in bf16 (PE fast-weight-load), PSUM/elementwise in fp32.
Sigmoid is computed via tanh with 0.5-prescaled rows (ACT does one tanh over
all 4 gate blocks); h is stored doubled (h' = 2h) with 0.5 folded into every
weight column that consumes h.
"""

import time

import numpy as np

import concourse.bass as bass
import concourse.mybir as mybir
import concourse.tile as tile
from concourse import bacc
from concourse.bass import ds

try:
    import ml_dtypes

    _BF16 = ml_dtypes.bfloat16
except Exception:  # pragma: no cover
    _BF16 = np.float32

T = 512
IN_DIM = 85
H = 1024
KC = H // 128            # 8 k-chunks
M = 4 * H                # 4096 gates
MT = M // 128            # 32 m-tiles
LAYERS = 3
VERTS = 6890
NOUT = VERTS * 3         # 20670
NCORES = 8
NSLICE = 2584            # padded per-core output slice (8*2584 = 20672)
B = 16                   # steps per block-slot
NB = T // B              # 32 blocks
NS = NB + 6              # slots: core p scans block s-(2p+1); mailbox read at slot end

f32 = mybir.dt.float32
bf16 = mybir.dt.bfloat16
i32 = mybir.dt.int32
AF = mybir.ActivationFunctionType
ALU = mybir.AluOpType


def build():
    nc = bacc.Bacc("TRN2", target_bir_lowering=False, debug=False)

    whh_d = nc.dram_tensor("whh", [128, KC * M], bf16, kind="ExternalInput")
    wih_d = nc.dram_tensor("wih", [128, KC * M], bf16, kind="ExternalInput")
    bias_d = nc.dram_tensor("bias", [128, MT], f32, kind="ExternalInput")
    pose_d = nc.dram_tensor("pose", [128, 8 * T], bf16, kind="ExternalInput")
    wout_d = nc.dram_tensor("wout", [128, KC * NSLICE], bf16, kind="ExternalInput")
    masks_d = nc.dram_tensor("masks", [128, 8], f32, kind="ExternalInput")
    out_d = nc.dram_tensor("out", [T, NSLICE], f32, kind="ExternalOutput")

    SCR = 8 * (NS * B + 1)   # h-history cols (slot-indexed, +1 for final h)

    with tile.TileContext(nc) as tc:
        with (
            tc.tile_pool(name="persist", bufs=1) as pers,
            tc.tile_pool(name="dram", bufs=1, space="DRAM") as dp,
        ):
            whh = pers.tile([128, KC * M], bf16)
            wih = pers.tile([128, KC * M], bf16)
            bia = pers.tile([128, MT], f32)
            msk = pers.tile([128, 8], f32)
            pose = pers.tile([128, 8 * T], bf16)
            nc.gpsimd.dma_start(whh[:], whh_d[:])
            nc.gpsimd.dma_start(wih[:], wih_d[:])
            nc.gpsimd.dma_start(bia[:], bias_d[:])
            nc.gpsimd.dma_start(msk[:], masks_d[:])
            nc.gpsimd.dma_start(pose[:], pose_d[:])

            hh = pers.tile([128, SCR], bf16)       # h' history, slot-indexed
            h2 = pers.tile([128, 8 * T], bf16)     # layer-2 h' history (all cores)
            hb = pers.tile([128, 16], bf16)        # ping-pong h' state (static addrs)
            cs = pers.tile([128, 8], f32)          # c' state
            zb = pers.tile([128, 8], bf16)
            zf = pers.tile([128, 8], f32)
            xgb = pers.tile([128, 32 * B], f32)    # current block's gate inputs
            nc.vector.memset(hh[:, 0:8], 0.0)
            nc.vector.memset(hb[:], 0.0)
            nc.vector.memset(xgb[:], 0.0)
            nc.vector.memset(cs[:], 0.0)
            nc.vector.memset(zb[:], 0.0)
            nc.vector.memset(zf[:], 0.0)

            # mailbox bounce buffers + DRAM zero sources for conditional resets
            obox = dp.tile([128, 8 * B], bf16)
            iboxes = {
                s: dp.tile(
                    [8 * 128, 8 * B], bf16, addr_space="Shared", name=f"ibox{s}"
                )
                for s in range(1, NS)
            }

            with (
                tc.tile_pool(name="stage", bufs=2) as stg,
                tc.tile_pool(name="ew", bufs=2) as ep,
                tc.tile_pool(name="psA", bufs=2, space="PSUM") as ppg,
                tc.tile_pool(name="psB", bufs=2, space="PSUM") as ppx,
            ):

                # m-tile order: element-halves first (h elems 0:512 then 512:1024)
                # so each half's elementwise overlaps the other half's matmuls
                loM = [m for m in range(MT) if (m % 8) < 4]
                hiM = [m for m in range(MT) if (m % 8) >= 4]
                MORD = loM + hiM

                def xg_gemm(st_ap):
                    """gate inputs for one block: xgb[:, t*32+j] (j in MORD order)."""
                    for j, m in enumerate(MORD):
                        px = ppx.tile([128, B], f32, tag="xg")
                        for k in range(KC):
                            nc.tensor.matmul(
                                px[:],
                                wih[:, k * M + m * 128 : k * M + (m + 1) * 128],
                                st_ap[:, ds(k, B, 8)],
                                start=(k == 0),
                                stop=(k == KC - 1),
                            )
                        nc.vector.tensor_scalar_add(
                            xgb[:, ds(j, B, 32)], px[:], bia[:, m : m + 1]
                        )

                def chain(ps, csl, hdst, xap, tg):
                    """gate nonlinearity for one element-half (16 ps cols: i,f,o,g x4)."""
                    pre = ep.tile([128, 16], f32, tag=f"pre{tg}")
                    nc.vector.tensor_add(pre[:], ps[:], xap)
                    th = ep.tile([128, 16], f32, tag=f"th{tg}")
                    nc.scalar.activation(th[:], pre[:], AF.Tanh)
                    A = ep.tile([128, 4], f32, tag=f"A{tg}")
                    nc.vector.scalar_tensor_tensor(
                        A[:], th[:, 4:8], 1.0, csl, ALU.add, ALU.mult
                    )
                    Bt = ep.tile([128, 4], f32, tag=f"B{tg}")
                    nc.vector.scalar_tensor_tensor(
                        Bt[:], th[:, 0:4], 1.0, th[:, 12:16], ALU.add, ALU.mult
                    )
                    nc.vector.scalar_tensor_tensor(
                        csl, A[:], 0.5, Bt[:], ALU.mult, ALU.add
                    )
                    tch = ep.tile([128, 4], f32, tag=f"tc{tg}")
                    nc.scalar.activation(tch[:], csl, AF.Tanh, scale=0.5)
                    nc.vector.scalar_tensor_tensor(
                        hdst, th[:, 8:12], 1.0, tch[:], ALU.add, ALU.mult
                    )

                def scan_pair(iv2, base):
                    """two LSTM steps (ping-pong through hb's static halves)."""
                    for half in range(2):
                        src = hb[:, half * 8 : half * 8 + 8]
                        dst = hb[:, (1 - half) * 8 : (1 - half) * 8 + 8]
                        psL = ppg.tile([128, 16], f32, tag="gl")
                        psH = ppg.tile([128, 16], f32, tag="gh")
                        for mi, m in enumerate(loM):
                            for k in range(KC):
                                nc.tensor.matmul(
                                    psL[:, mi : mi + 1],
                                    whh[:, k * M + m * 128 : k * M + (m + 1) * 128],
                                    src[:, k : k + 1],
                                    start=(k == 0),
                                    stop=(k == KC - 1),
                                )
                        for mi, m in enumerate(hiM):
                            for k in range(KC):
                                nc.tensor.matmul(
                                    psH[:, mi : mi + 1],
                                    whh[:, k * M + m * 128 : k * M + (m + 1) * 128],
                                    src[:, k : k + 1],
                                    start=(k == 0),
                                    stop=(k == KC - 1),
                                )
                        xo = iv2 * 64 + half * 32
                        chain(psL[:], cs[:, 0:4], dst[:, 0:4],
                              xgb[:, ds(xo, 16)], "L")
                        chain(psH[:], cs[:, 4:8], dst[:, 4:8],
                              xgb[:, ds(xo + 16, 16)], "H")
                        # history for mailbox/output, off the critical path
                        nc.scalar.copy(
                            hh[:, ds(iv2 * 16 + half * 8 + base + 8, 8)], dst
                        )

                for s in range(NS):
                    if s >= 1:
                        # ship previous slot's products; consumers read this
                        # mailbox only at the END of this slot, so the CC
                        # completes under the scan and nobody waits on it
                        nc.gpsimd.dma_start(
                            obox[:], hh[:, (s - 1) * B * 8 + 8 : s * B * 8 + 8]
                        )
                        nc.gpsimd.collective_compute(
                            "AllGather",
                            ALU.bypass,
                            replica_groups=[list(range(NCORES))],
                            ins=[obox[:].opt()],
                            outs=[iboxes[s][:].opt()],
                        )
                    # state resets just before each core's first valid block
                    # (core p scans block 0 at slot 2p+1)
                    if s in (1, 3, 5):
                        km = {1: msk[:, 4:5], 3: msk[:, 5:6], 5: msk[:, 6:7]}[s]
                        nc.vector.tensor_scalar_mul(cs[:], cs[:], km)
                        nc.vector.tensor_scalar_mul(
                            hb[:, 0:8], hb[:, 0:8], km,
                        )
                    # scan this slot's block with xgb prepared last slot
                    if s <= NB + 4:
                        base = s * B * 8
                        with tc.For_i(0, B // 2, 1, hint_engines=(mybir.EngineType.PE,), staggered_reset=True) as iv2:
                            scan_pair(iv2, base)
                    # collect layer-2 h' blocks for the output GEMM
                    if 6 <= s <= NB + 5:
                        nc.gpsimd.dma_start(
                            h2[:, (s - 6) * B * 8 : (s - 5) * B * 8],
                            iboxes[s][2 * 128 : 3 * 128, :],
                        )
                    # prepare NEXT slot's gate inputs from this slot's mailbox
                    if s <= NB + 3:
                        st = stg.tile([128, 8 * B], bf16, tag="st")
                        if s >= 1:
                            r0 = stg.tile([128, 8 * B], bf16, tag="r0")
                            r1 = stg.tile([128, 8 * B], bf16, tag="r1")
                            nc.gpsimd.dma_start(r0[:], iboxes[s][0:128, :])
                            nc.gpsimd.dma_start(r1[:], iboxes[s][128:256, :])
                            nc.vector.tensor_scalar_mul(st[:], r0[:], msk[:, 0:1])
                            nc.vector.scalar_tensor_tensor(
                                st[:], r1[:], msk[:, 1:2], st[:], ALU.mult, ALU.add
                            )
                            if s < NB:
                                nc.vector.scalar_tensor_tensor(
                                    st[:], pose[:, s * 8 * B : (s + 1) * 8 * B],
                                    msk[:, 3:4], st[:], ALU.mult, ALU.add,
                                )
                        else:
                            nc.vector.tensor_scalar_mul(
                                st[:], pose[:, 0 : 8 * B], msk[:, 3:4]
                            )
                        xg_gemm(st)

            # ---- output GEMM: out[t, n] = h2'[1..T] @ wout_eff^T ----
            with (
                tc.tile_pool(name="og_sb", bufs=3) as osb,
                tc.tile_pool(name="og_ps", bufs=2, space="PSUM") as ops,
            ):
                nchunks = [
                    (i * 512, min(512, NSLICE - i * 512))
                    for i in range((NSLICE + 511) // 512)
                ]
                for n0, nsz in nchunks:
                    psl = [
                        ops.tile([128, 512], f32, tag=f"og{tt}", name=f"og{tt}")
                        for tt in range(T // 128)
                    ]
                    for k in range(KC):
                        w = osb.tile([128, 512], bf16, tag="wo")
                        nc.gpsimd.dma_start(
                            w[:, :nsz],
                            wout_d[:, k * NSLICE + n0 : k * NSLICE + n0 + nsz],
                        )
                        for tt in range(T // 128):
                            nc.tensor.matmul(
                                psl[tt][:, :nsz],
                                h2[:, ds(tt * 1024 + k, 128, 8)],
                                w[:, :nsz],
                                start=(k == 0),
                                stop=(k == KC - 1),
                            )
                    for tt in range(T // 128):
                        ot = osb.tile([128, 512], f32, tag="ot")
                        nc.scalar.copy(ot[:, :nsz], psl[tt][:, :nsz])
                        nc.gpsimd.dma_start(
                            out_d[tt * 128 : (tt + 1) * 128, n0 : n0 + nsz],
                            ot[:, :nsz],
                        )

    nc.compile()
    return nc


def build_baseline():
    """Same I/O signature, near-empty body — measures dispatch overhead."""
    nc = bacc.Bacc("TRN2", target_bir_lowering=False, debug=False)
    nc.dram_tensor("whh", [128, KC * M], bf16, kind="ExternalInput")
    nc.dram_tensor("wih", [128, KC * M], bf16, kind="ExternalInput")
    nc.dram_tensor("bias", [128, MT], f32, kind="ExternalInput")
    nc.dram_tensor("pose", [128, 8 * T], bf16, kind="ExternalInput")
    nc.dram_tensor("wout", [128, KC * NSLICE], bf16, kind="ExternalInput")
    masks_d = nc.dram_tensor("masks", [128, 8], f32, kind="ExternalInput")
    out_d = nc.dram_tensor("out", [T, NSLICE], f32, kind="ExternalOutput")
    with tile.TileContext(nc) as tc:
        with tc.tile_pool(name="sb", bufs=1) as sb:
            t = sb.tile([128, 8], f32)
            nc.gpsimd.dma_start(t[:], masks_d[:])
            nc.gpsimd.dma_start(out_d[0:128, 0:8], t[:])
    nc.compile()
    return nc


_CACHE = {}


def _prep_core_inputs(inputs):
    """Slice/scale/reorder reference weights into per-core device layouts."""
    pose_np = np.asarray(inputs["pose_beta_seq"], np.float32)      # [T, 85]
    w_out = np.asarray(inputs["W_out"], np.float32)                # [20670, H]

    def to_kc_m(WT, m_cols):
        # WT: [H, m_cols] -> [128, KC*m_cols]
        o = np.empty((128, KC * m_cols), np.float32)
        for k in range(KC):
            o[:, k * m_cols : (k + 1) * m_cols] = WT[k * 128 : (k + 1) * 128, :]
        return o

    def reorder_rows(Wr):
        i, f, g, o = np.split(Wr, 4, axis=0)
        return np.concatenate([i, f, o, g], axis=0)

    rowscale = np.ones((M, 1), np.float32)
    rowscale[: 3 * H] = 0.5      # i,f,o rows -> tanh-as-sigmoid

    per_core = []
    for p in range(NCORES):
        d = {}
        if p < LAYERS:
            Whh = reorder_rows(np.asarray(inputs[f"W_hh_{p}"], np.float32))
            Wih = reorder_rows(np.asarray(inputs[f"W_ih_{p}"], np.float32))
            bi = np.asarray(inputs[f"b_ih_{p}"], np.float32) + np.asarray(
                inputs[f"b_hh_{p}"], np.float32
            )
            bi = reorder_rows(bi.reshape(M, 1))
            in_dim = Wih.shape[1]
            Wih_p = np.zeros((M, H), np.float32)
            colscale = 0.5 if p > 0 else 1.0     # inputs are h' = 2h for l>0
            Wih_p[:, :in_dim] = Wih * colscale
            whh_eff = Whh * rowscale * 0.5       # own-state h' = 2h
            wih_eff = Wih_p * rowscale
            b_eff = bi * rowscale
            d["whh"] = to_kc_m(whh_eff.T, M).astype(_BF16)
            d["wih"] = to_kc_m(wih_eff.T, M).astype(_BF16)
            d["bias"] = b_eff.reshape(MT, 128).T.astype(np.float32)
        else:
            d["whh"] = np.zeros((128, KC * M), _BF16)
            d["wih"] = np.zeros((128, KC * M), _BF16)
            d["bias"] = np.zeros((128, MT), np.float32)
        if p == 0:
            pT = np.zeros((H, T), np.float32)
            pT[:IN_DIM, :] = pose_np.T
            po = pT.reshape(KC, 128, T).transpose(1, 2, 0).reshape(128, 8 * T)
            d["pose"] = po.astype(_BF16)
        else:
            d["pose"] = np.zeros((128, 8 * T), _BF16)
        wo = np.zeros((NSLICE, H), np.float32)
        lo = p * NSLICE
        hi = min(NOUT, lo + NSLICE)
        if hi > lo:
            wo[: hi - lo] = w_out[lo:hi]
        d["wout"] = to_kc_m((wo * 0.5).T, NSLICE).astype(_BF16)
        mk = np.zeros((8,), np.float32)
        mk[0] = 1.0 if p == 1 else 0.0   # use mailbox rank 0
        mk[1] = 1.0 if p == 2 else 0.0   # use mailbox rank 1
        mk[3] = 1.0 if p == 0 else 0.0   # use pose
        mk[4] = 0.0 if p == 0 else 1.0   # reset at slot 1 (core 0)
        mk[5] = 0.0 if p == 1 else 1.0   # reset at slot 3 (core 1)
        mk[6] = 0.0 if p == 2 else 1.0   # reset at slot 5 (core 2)
        d["masks"] = np.tile(mk[None, :], (128, 1))
        per_core.append(d)
    return per_core


def _get_runner(which="full"):
    key = f"runner_{which}"
    if key in _CACHE:
        return _CACHE[key]
    import jax
    from jax.sharding import Mesh, PartitionSpec
    from jax.experimental.shard_map import shard_map
    from concourse import bass2jax
    from concourse.bass2jax import _bass_exec_p, partition_id_tensor

    nc = build() if which == "full" else build_baseline()
    bass2jax.install_neuronx_cc_hook()

    in_names, out_names, out_avals, zero_outs = [], [], [], []
    partition_name = nc.partition_id_tensor.name if nc.partition_id_tensor else None
    for alloc in nc.m.functions[0].allocations:
        if not isinstance(alloc, mybir.MemoryLocationSet):
            continue
        name = alloc.memorylocations[0].name
        if alloc.kind == "ExternalInput":
            if name != partition_name:
                in_names.append(name)
        elif alloc.kind == "ExternalOutput":
            out_names.append(name)
            shape = tuple(alloc.tensor_shape)
            dtype = mybir.dt.np(alloc.dtype)
            out_avals.append(jax.core.ShapedArray(shape, dtype))
            zero_outs.append(np.zeros(shape, dtype))
    n_params = len(in_names)
    all_in = list(in_names) + list(out_names)
    if partition_name is not None:
        all_in.append(partition_name)

    def _body(*args):
        operands = list(args)
        if partition_name is not None:
            operands.append(partition_id_tensor())
        outs = _bass_exec_p.bind(
            *operands,
            out_avals=tuple(out_avals),
            in_names=tuple(all_in),
            out_names=tuple(out_names),
            lowering_input_output_aliases=(),
            sim_require_finite=True,
            sim_require_nnan=True,
            nc=nc,
        )
        return tuple(outs)

    devices = jax.devices()[:NCORES]
    mesh = Mesh(np.asarray(devices), ("core",))
    nouts = len(out_names)
    sharded = jax.jit(
        shard_map(
            _body,
            mesh=mesh,
            in_specs=(PartitionSpec("core"),) * (n_params + nouts),
            out_specs=(PartitionSpec("core"),) * nouts,
            check_rep=False,
        ),
        donate_argnums=tuple(range(n_params, n_params + nouts)),
    )

    def run(per_core):
        gin = [
            np.concatenate([np.asarray(pc[nm]) for pc in per_core], axis=0)
            for nm in in_names
        ]
        gzo = [np.concatenate([z] * NCORES, axis=0) for z in zero_outs]
        sh = jax.sharding.NamedSharding(mesh, PartitionSpec("core"))
        gin = [jax.device_put(a, sh) for a in gin]
        gzo = [jax.device_put(a, sh) for a in gzo]
        jax.block_until_ready(gin)
        t0 = time.time()
        outs = sharded(*gin, *gzo)
        jax.block_until_ready(outs)
        _CACHE[f"exec_wall_s_{which}"] = time.time() - t0
        res = [np.asarray(o) for o in outs]
        per_core_out = []
        for c in range(NCORES):
            per_core_out.append(
                {
                    nm: res[i][
                        c * zero_outs[i].shape[0] : (c + 1) * zero_outs[i].shape[0]
                    ]
                    for i, nm in enumerate(out_names)
                }
            )
        return per_core_out

    _CACHE[key] = run
    return run


def kernel(**inputs):
    run = _get_runner("full")
    per_core = _prep_core_inputs(inputs)
    t0 = time.time()
    outs = run(per_core)
    _CACHE["last_wall_s"] = time.time() - t0
    _CACHE["exec_wall_s"] = _CACHE.get("exec_wall_s_full")
    full = np.concatenate([outs[c]["out"] for c in range(NCORES)], axis=1)[:, :NOUT]
    return full.reshape(T, VERTS, 3).astype(np.float32)

